# revision 51
# baseline (speedup 1.0000x reference)
"""Trainium2 Bass kernel for DeformableSentenceSplit.

Math (per batch row):
    pooled  = mean(x, axis=T)                       # [E]
    off     = pooled @ W + b                        # [2S]
    s_off   = floor(clip(off[:S], 0, L-1))          # [S]
    e_off   = floor(clip(off[S:], 0, L-1))          # [S]
    start   = min(s*L + s_off, T-L)                 # [S]
    end     = min(max(s*L + L + e_off, start), T)   # [S]
    out[s, j, :] = x[start_s + j, :] if j < end_s - start_s else 0

Key properties exploited:
  - start_s + j <= T-1 always, so each sentence is a contiguous 64-row
    block of the input -> indirect (gather) DMA with one row index per
    output partition.
  - The output is a masked permutation of input rows: values are copied
    bit-exactly; only the index computation involves arithmetic.

Sharding: pure data parallel. 16 batch rows over 8 cores = 2 rows/core.
W/b and small constant matrices are replicated.
"""

import sys

for _p in ("/opt/trn_rl_repo",):
    if _p not in sys.path:
        sys.path.insert(0, _p)

import numpy as np

import concourse.bass as bass
import concourse.mybir as mybir
import concourse.tile as tile
from concourse import bacc, bass_utils
from concourse.bass import IndirectOffsetOnAxis

# Problem shape (hardcoded per contract).
B = 16
S = 32          # sentences
L = 64          # max sentence length
E = 768         # embed dim
T = S * L       # 2048 text length
NCORES = 8
B_LOC = B // NCORES     # 2 batch rows per core
ROWS = B_LOC * T        # 4096 input rows per core
NTILES = T // 128       # 16 x-tiles of 128 rows per batch row
NGROUPS = T // 128      # 16 output groups of 128 rows (2 sentences) per batch row
TRASH = 256             # scat2: spare out rows absorbing invalid-slot writes

FP32 = mybir.dt.float32
I32 = mybir.dt.int32
Alu = mybir.AluOpType


def _host_constants():
    """Small constant tensors replicated to every core."""
    p = np.arange(128)
    s = np.arange(S)
    # selmat[s, p] = 1 iff sentence s lands on partition half p//64 of its
    # group: out partition p of group g holds sentence 2g + p//64.
    selmat = (s[:, None] % 2 == p[None, :] // 64).astype(np.float32)
    # gmask[s, g] = 1 iff sentence s belongs to group g = s//2.
    gmask = (s[:, None] // 2 == np.arange(NGROUPS)[None, :]).astype(np.float32)
    jpat = (p % 64).astype(np.float32).reshape(128, 1)      # j within sentence
    basev = (s * L).astype(np.float32).reshape(S, 1)        # s*L
    invt = np.full((128, 1), 1.0 / T, dtype=np.float32)
    # kmat[s, k] = k+1 for k in [0, L-2]; floor(x) = sum_k (x >= k+1) on [0, L-1]
    kmat = np.tile(np.arange(1, L, dtype=np.float32), (S, 1))
    # scatter-path constants: input row t = 128*i + p of tile i belongs to
    # sentence s1 = t//64 (pass 1) or s2 = t//64 - 1 (pass 2).
    i_idx = np.arange(NTILES)
    tpat = (128 * i_idx[None, :] + p[:, None]).astype(np.float32)       # [128, 16]
    srow1 = (128 * i_idx[None, :] + 64 * (p[:, None] // 64)).astype(np.float32)
    sel2 = (s[:, None] % 2 == 1 - p[None, :] // 64).astype(np.float32)  # [32, 128]
    gmask2 = ((s[:, None] + 1) // 2 == i_idx[None, :]).astype(np.float32)
    # scat2 (dma_scatter_add) constants, index position i = 16*m + q on a
    # [16 partitions, 128 cols] layout; x row t = i; sentence s1 = m//4.
    q16 = np.arange(16)[:, None]
    m16 = np.arange(128)[None, :]
    tpat16 = (16 * m16 + q16).astype(np.float32)             # [16, 128] = t
    srow16 = (64 * (m16 // 4) + 0 * q16).astype(np.float32)  # [16, 128] = 64*s1
    trash16 = (T + (16 * m16 + q16) % TRASH).astype(np.float32)
    id32 = np.eye(S, dtype=np.float32)
    return (
        selmat, gmask, jpat, basev, invt, kmat, tpat, srow1, sel2, gmask2,
        tpat16, srow16, trash16, id32,
    )


def build_nc(debug: bool = False, variant: str = "full"):
    # variant: bisect knob - "full" | "nomask" (skip mask multiply)
    #          | "nogather" (plain copy instead of indirect DMA)
    nc = bacc.Bacc("TRN2", target_bir_lowering=False, debug=debug)

    x = nc.dram_tensor("x", [ROWS, E], FP32, kind="ExternalInput").ap()
    w = nc.dram_tensor("w", [E, 2 * S], FP32, kind="ExternalInput").ap()
    b_s = nc.dram_tensor("b_s", [S, 1], FP32, kind="ExternalInput").ap()
    b_e = nc.dram_tensor("b_e", [S, 1], FP32, kind="ExternalInput").ap()
    selmat = nc.dram_tensor("selmat", [S, 128], FP32, kind="ExternalInput").ap()
    gmask = nc.dram_tensor("gmask", [S, NGROUPS], FP32, kind="ExternalInput").ap()
    jpat = nc.dram_tensor("jpat", [128, 1], FP32, kind="ExternalInput").ap()
    basev = nc.dram_tensor("basev", [S, 1], FP32, kind="ExternalInput").ap()
    invt = nc.dram_tensor("invt", [128, 1], FP32, kind="ExternalInput").ap()
    kmat = nc.dram_tensor("kmat", [S, L - 1], FP32, kind="ExternalInput").ap()
    tpat = nc.dram_tensor("tpat", [128, NTILES], FP32, kind="ExternalInput").ap()
    srow1 = nc.dram_tensor("srow1", [128, NTILES], FP32, kind="ExternalInput").ap()
    sel2 = nc.dram_tensor("sel2", [S, 128], FP32, kind="ExternalInput").ap()
    gmask2 = nc.dram_tensor("gmask2", [S, NTILES], FP32, kind="ExternalInput").ap()
    tpat16 = nc.dram_tensor("tpat16", [16, 128], FP32, kind="ExternalInput").ap()
    srow16 = nc.dram_tensor("srow16", [16, 128], FP32, kind="ExternalInput").ap()
    trash16 = nc.dram_tensor("trash16", [16, 128], FP32, kind="ExternalInput").ap()
    id32 = nc.dram_tensor("id32", [S, S], FP32, kind="ExternalInput").ap()
    if variant.startswith("scat"):
        # one output tensor per batch row: the two rows' scatters have no
        # WAW overlap, so Tile lets them pipeline instead of serializing.
        # scat2 pads each with TRASH rows that absorb invalid-slot writes.
        pad = TRASH if variant.startswith("scat2") else 0
        outs = [
            nc.dram_tensor(f"out{b}", [T + pad, E], FP32, kind="ExternalOutput").ap()
            for b in range(B_LOC)
        ]
        out = None
    else:
        out = nc.dram_tensor("out", [ROWS, E], FP32, kind="ExternalOutput").ap()
    if variant == "scatterdbg":
        dbg = nc.dram_tensor("dbg", [128, 4 * NTILES], I32, kind="ExternalOutput").ap()

    with tile.TileContext(nc) as tc:
        with (
            tc.tile_pool(name="xin", bufs=4) as xpool,
            tc.tile_pool(name="gath", bufs=4) as gpool,
            tc.tile_pool(name="small", bufs=1) as spool,
            tc.tile_pool(name="psum", bufs=1, space="PSUM") as ppool,
        ):
            # ---- constants into SBUF ----
            w_sb = spool.tile([128, 6 * 2 * S], FP32, tag="w_sb")
            nc.sync.dma_start(
                out=w_sb[:].rearrange("p (c n) -> p c n", c=6),
                in_=w.rearrange("(c p) n -> p c n", p=128),
            )
            bs_sb = spool.tile([S, 1], FP32, tag="bs")
            nc.sync.dma_start(out=bs_sb[:], in_=b_s)
            be_sb = spool.tile([S, 1], FP32, tag="be")
            nc.sync.dma_start(out=be_sb[:], in_=b_e)
            sel_sb = spool.tile([S, 128], FP32, tag="sel")
            nc.sync.dma_start(out=sel_sb[:], in_=selmat)
            gm_sb = spool.tile([S, NGROUPS], FP32, tag="gm")
            nc.sync.dma_start(out=gm_sb[:], in_=gmask)
            jp_sb = spool.tile([128, 1], FP32, tag="jp")
            nc.sync.dma_start(out=jp_sb[:], in_=jpat)
            base_sb = spool.tile([S, 1], FP32, tag="base")
            nc.sync.dma_start(out=base_sb[:], in_=basev)
            invt_sb = spool.tile([128, 1], FP32, tag="invt")
            nc.sync.dma_start(out=invt_sb[:], in_=invt)
            km_sb = spool.tile([S, L - 1], FP32, tag="km")
            nc.sync.dma_start(out=km_sb[:], in_=kmat)
            tp_sb = spool.tile([128, NTILES], FP32, tag="tp")
            nc.sync.dma_start(out=tp_sb[:], in_=tpat)
            sr_sb = spool.tile([128, NTILES], FP32, tag="sr")
            nc.sync.dma_start(out=sr_sb[:], in_=srow1)
            sel2_sb = spool.tile([S, 128], FP32, tag="sel2")
            nc.sync.dma_start(out=sel2_sb[:], in_=sel2)
            gm2_sb = spool.tile([S, NTILES], FP32, tag="gm2")
            nc.sync.dma_start(out=gm2_sb[:], in_=gmask2)
            tp16_sb = spool.tile([16, 128], FP32, tag="tp16")
            nc.sync.dma_start(out=tp16_sb[:], in_=tpat16)
            sr16_sb = spool.tile([16, 128], FP32, tag="sr16")
            nc.sync.dma_start(out=sr16_sb[:], in_=srow16)
            tr16_sb = spool.tile([16, 128], FP32, tag="tr16")
            nc.sync.dma_start(out=tr16_sb[:], in_=trash16)
            id32_sb = spool.tile([S, S], FP32, tag="id32")
            nc.sync.dma_start(out=id32_sb[:], in_=id32)
            ones16_sb = spool.tile([1, 16], FP32, tag="ones16")
            nc.vector.memset(ones16_sb[:], 1.0)

            scatter = variant.startswith("scat")
            if scatter:
                # all 32 x-tiles resident in one big SBUF tile (12.6 MB):
                # column block k = 16*b + i holds input rows [128k, 128k+128)
                xbig = spool.tile([128, B_LOC * NTILES * E], FP32, tag="xbig")
                for k in range(B_LOC * NTILES):
                    nc.sync.dma_start(
                        out=xbig[:, k * E : (k + 1) * E],
                        in_=x[128 * k : 128 * (k + 1), :],
                    )

            upto = 99
            if variant.startswith("upto"):
                upto = int(variant[4:])
            for b in range(B_LOC):
                # ---- phase 1: partial sums over T (DVE), then partition
                # reduce + scale via matmul -> pooled^T chunks [128, 6] ----
                acc = spool.tile([128, E], FP32, tag=f"acc{b}")
                for t in range(NTILES):
                    if scatter:
                        k = b * NTILES + t
                        xt_ap = xbig[:, k * E : (k + 1) * E]
                    else:
                        xt = xpool.tile([128, E], FP32, tag="xt")
                        nc.sync.dma_start(
                            out=xt[:],
                            in_=x[b * T + t * 128 : b * T + (t + 1) * 128, :],
                        )
                        xt_ap = xt[:]
                    if t == 0:
                        nc.vector.tensor_copy(out=acc[:], in_=xt_ap)
                    else:
                        nc.vector.tensor_tensor(
                            out=acc[:], in0=acc[:], in1=xt_ap, op=Alu.add
                        )

                poolT_ps = ppool.tile([128, 6], FP32, tag="poolT")
                for e in range(6):
                    nc.tensor.matmul(
                        out=poolT_ps[:, e : e + 1],
                        lhsT=acc[:, e * 128 : (e + 1) * 128],
                        rhs=invt_sb[:],
                        start=True,
                        stop=True,
                    )
                poolT_sb = spool.tile([128, 6], FP32, tag=f"poolTsb{b}")
                nc.vector.tensor_copy(out=poolT_sb[:], in_=poolT_ps[:])
                if upto <= 1:
                    continue

                # ---- phase 2: offsets = pooled @ W + b, split start/end ----
                offS_ps = ppool.tile([S, 1], FP32, tag="offS")
                offE_ps = ppool.tile([S, 1], FP32, tag="offE")
                for e in range(6):
                    nc.tensor.matmul(
                        out=offS_ps[:],
                        lhsT=w_sb[:, 64 * e : 64 * e + 32],
                        rhs=poolT_sb[:, e : e + 1],
                        start=(e == 0),
                        stop=(e == 5),
                    )
                for e in range(6):
                    nc.tensor.matmul(
                        out=offE_ps[:],
                        lhsT=w_sb[:, 64 * e + 32 : 64 * e + 64],
                        rhs=poolT_sb[:, e : e + 1],
                        start=(e == 0),
                        stop=(e == 5),
                    )

                offS = spool.tile([S, 1], FP32, tag=f"offSsb{b}")
                nc.vector.tensor_tensor(
                    out=offS[:], in0=offS_ps[:], in1=bs_sb[:], op=Alu.add
                )
                offE = spool.tile([S, 1], FP32, tag=f"offEsb{b}")
                nc.vector.tensor_tensor(
                    out=offE[:], in0=offE_ps[:], in1=be_sb[:], op=Alu.add
                )
                if upto <= 2:
                    continue

                # ---- phase 3: index math on [S, 1] tiles ----
                # clip to [0, L-1], then floor(x) = sum_k (x >= k), exact in fp32
                def clip_floor(src, tag):
                    c = spool.tile([S, 1], FP32, tag=f"{tag}c{b}")
                    nc.vector.tensor_scalar(
                        out=c[:], in0=src[:], scalar1=0.0, scalar2=float(L - 1),
                        op0=Alu.max, op1=Alu.min,
                    )
                    if variant == "nottr":
                        return c
                    scratch = spool.tile([S, L - 1], FP32, tag=f"{tag}scr{b}")
                    nc.vector.tensor_tensor(
                        out=scratch[:],
                        in0=c[:].to_broadcast([S, L - 1]),
                        in1=km_sb[:],
                        op=Alu.is_ge,
                    )
                    o = spool.tile([S, 1], FP32, tag=f"{tag}o{b}")
                    nc.vector.reduce_sum(
                        out=o[:], in_=scratch[:], axis=mybir.AxisListType.X
                    )
                    return o

                s_off = clip_floor(offS, "s")
                e_off = clip_floor(offE, "e")

                # start = min(base + s_off, T - L)
                start = spool.tile([S, 1], FP32, tag=f"start{b}")
                nc.vector.tensor_tensor(
                    out=start[:], in0=base_sb[:], in1=s_off[:], op=Alu.add
                )
                nc.vector.tensor_scalar(
                    out=start[:], in0=start[:], scalar1=float(T - L), scalar2=None,
                    op0=Alu.min,
                )
                # end = min(max(base + L + e_off, start), T)
                end = spool.tile([S, 1], FP32, tag=f"end{b}")
                nc.vector.tensor_tensor(
                    out=end[:], in0=base_sb[:], in1=e_off[:], op=Alu.add
                )
                nc.vector.tensor_scalar(
                    out=end[:], in0=end[:], scalar1=float(L), scalar2=None, op0=Alu.add
                )
                nc.vector.tensor_tensor(
                    out=end[:], in0=end[:], in1=start[:], op=Alu.max
                )
                nc.vector.tensor_scalar(
                    out=end[:], in0=end[:], scalar1=float(T), scalar2=None, op0=Alu.min
                )
                # nvalid = end - start
                nv = spool.tile([S, 1], FP32, tag=f"nv{b}")
                nc.vector.tensor_tensor(
                    out=nv[:], in0=end[:], in1=start[:], op=Alu.subtract
                )
                if upto <= 3:
                    continue

                if scatter:
                    # ---- scatter path: write each resident input row to its
                    # output position(s); masked tails stay zero (pre-zeroed
                    # out). pass 1: row t -> sentence t//64; pass 2: t//64 - 1.
                    nv64 = spool.tile([S, 1], FP32, tag=f"nv64{b}")
                    nc.vector.tensor_scalar(
                        out=nv64[:], in0=nv[:], scalar1=float(L), scalar2=None,
                        op0=Alu.min,
                    )

                if variant.startswith("scat2"):
                    # ---- dma_scatter_add path. Index position i = 16m + q on
                    # a [16, 128] tile maps to x row t = i; its sentence
                    # s1 = m//4 (pass 1) or s1 - 1 (pass 2) depends only on m.
                    # Invalid mid-stream slots -> trash rows (>= T); slots
                    # after the last valid one -> -1 with num_idxs_reg
                    # truncating the transfer (a pass with no valid slots
                    # costs ~nothing).
                    # rows_sb[0, 0] = 0 is the s=-1 sentinel (nv 0 kills it);
                    # cols 1..32 = start_s, col 34 = sentinel, 35..66 = nv64_s.
                    rowT_ps = ppool.tile([1, 2 * S], FP32, tag="rowT")
                    nc.tensor.transpose(
                        out=rowT_ps[0:1, 0:S], in_=start[:], identity=id32_sb[:]
                    )
                    nc.tensor.transpose(
                        out=rowT_ps[0:1, S : 2 * S], in_=nv64[:], identity=id32_sb[:]
                    )
                    rows_sb = spool.tile([1, 70], FP32, tag="rows")
                    nc.vector.memset(rows_sb[:], 0.0)
                    nc.vector.tensor_copy(
                        out=rows_sb[0:1, 1 : S + 1], in_=rowT_ps[0:1, 0:S]
                    )
                    nc.vector.tensor_copy(
                        out=rows_sb[0:1, S + 3 : 2 * S + 3],
                        in_=rowT_ps[0:1, S : 2 * S],
                    )
                    # broadcast start/nv64 to [16, 128] via ones-matmul; the
                    # rhs AP expands (quantity 2, u 32, w 4-stride-0) from the
                    # row buffer at offset 1-pss (pass 2 reads s-1, hitting
                    # the zero sentinel for s=-1).
                    bc_ps = ppool.tile([16, 512], FP32, tag="bc")
                    for pss in (0, 1):
                        r0 = rows_sb[0:1, (1 - pss) : (2 - pss)]
                        rhs_ap = bass.AP(
                            r0.tensor,
                            r0.offset,
                            [r0.ap[0], [S + 2, 2], [1, S], [0, 4]],
                        )
                        nc.tensor.matmul(
                            out=bc_ps[0:16, 256 * pss : 256 * (pss + 1)],
                            lhsT=ones16_sb[:],
                            rhs=rhs_ap,
                            start=True,
                            stop=True,
                        )
                    for pss in (0, 1):
                        startB = bc_ps[0:16, 256 * pss : 256 * pss + 128]
                        nvB = bc_ps[0:16, 256 * pss + 128 : 256 * pss + 256]
                        jt = spool.tile([16, 128], FP32, tag=f"jt2_{b}{pss}")
                        nc.vector.tensor_tensor(
                            out=jt[:], in0=tp16_sb[:], in1=startB, op=Alu.subtract
                        )
                        vp = spool.tile([16, 128], FP32, tag=f"vp2_{b}{pss}")
                        nc.vector.tensor_scalar(
                            out=vp[:], in0=jt[:], scalar1=0.0, scalar2=None,
                            op0=Alu.is_ge,
                        )
                        v = spool.tile([16, 128], FP32, tag=f"v2_{b}{pss}")
                        nc.vector.tensor_tensor(
                            out=v[:], in0=jt[:], in1=nvB, op=Alu.is_lt
                        )
                        nc.vector.tensor_tensor(
                            out=v[:], in0=v[:], in1=vp[:], op=Alu.mult
                        )
                        outr = spool.tile([16, 128], FP32, tag=f"or2_{b}{pss}")
                        nc.vector.tensor_tensor(
                            out=outr[:], in0=jt[:], in1=sr16_sb[:], op=Alu.add
                        )
                        if pss:
                            nc.vector.tensor_scalar(
                                out=outr[:], in0=outr[:], scalar1=-64.0,
                                scalar2=None, op0=Alu.add,
                            )
                        # blend valid -> outr, invalid -> trash row
                        nc.vector.tensor_tensor(
                            out=outr[:], in0=outr[:], in1=tr16_sb[:], op=Alu.subtract
                        )
                        nc.vector.tensor_tensor(
                            out=outr[:], in0=outr[:], in1=v[:], op=Alu.mult
                        )
                        nc.vector.tensor_tensor(
                            out=outr[:], in0=outr[:], in1=tr16_sb[:], op=Alu.add
                        )
                        # lastpos = max_i (i+1)*valid_i
                        w = spool.tile([16, 128], FP32, tag=f"w2_{b}{pss}")
                        nc.vector.tensor_scalar(
                            out=w[:], in0=tp16_sb[:], scalar1=1.0, scalar2=None,
                            op0=Alu.add,
                        )
                        nc.vector.tensor_tensor(
                            out=w[:], in0=w[:], in1=v[:], op=Alu.mult
                        )
                        red1 = spool.tile([16, 1], FP32, tag=f"red1_{b}{pss}")
                        nc.vector.reduce_max(
                            out=red1[:], in_=w[:], axis=mybir.AxisListType.X
                        )
                        lpT_ps = ppool.tile([1, 16], FP32, tag="lpT")
                        nc.tensor.transpose(
                            out=lpT_ps[:], in_=red1[:], identity=id32_sb[0:16, 0:16]
                        )
                        lp = spool.tile([1, 1], FP32, tag=f"lp_{b}{pss}")
                        nc.vector.reduce_max(
                            out=lp[:], in_=lpT_ps[:], axis=mybir.AxisListType.X
                        )
                        # tail slots (i >= lastpos) -> -1
                        lpB_ps = ppool.tile([16, 1], FP32, tag="lpB")
                        nc.tensor.matmul(
                            out=lpB_ps[:], lhsT=ones16_sb[:], rhs=lp[:],
                            start=True, stop=True,
                        )
                        m1 = spool.tile([16, 128], FP32, tag=f"m1_{b}{pss}")
                        nc.vector.tensor_tensor(
                            out=m1[:],
                            in0=tp16_sb[:],
                            in1=lpB_ps[:].to_broadcast([16, 128]),
                            op=Alu.is_lt,
                        )
                        nc.vector.tensor_tensor(
                            out=outr[:], in0=outr[:], in1=m1[:], op=Alu.mult
                        )
                        nc.vector.tensor_scalar(
                            out=m1[:], in0=m1[:], scalar1=-1.0, scalar2=None,
                            op0=Alu.add,
                        )
                        nc.vector.tensor_tensor(
                            out=outr[:], in0=outr[:], in1=m1[:], op=Alu.add
                        )
                        idx16 = spool.tile(
                            [128, 128], mybir.dt.int16, tag=f"idx16_{b}{pss}"
                        )
                        nc.vector.memset(idx16[:], 0)
                        nc.vector.tensor_copy(out=idx16[0:16, :], in_=outr[:])
                        lpI = spool.tile([1, 1], I32, tag=f"lpI_{b}{pss}")
                        nc.vector.tensor_copy(out=lpI[:], in_=lp[:])
                        in3d = xbig[
                            :, b * NTILES * E : (b + 1) * NTILES * E
                        ].rearrange("p (c e) -> p c e", e=E)
                        reg = nc.gpsimd.value_load(
                            lpI[0:1, 0:1], min_val=0, max_val=T
                        )
                        nc.gpsimd.dma_scatter_add(
                            out_ap=outs[b],
                            in_ap=in3d,
                            idxs_ap=idx16[:],
                            num_idxs=T,
                            num_idxs_reg=reg,
                            elem_size=E,
                        )
                    continue

                if scatter:
                    # ---- indirect-DMA scatter variant: matches the sim but
                    # NOT the HW SWDGE descriptor walk; kept for reference ----
                    rhs3 = spool.tile([S, 4 * NTILES], FP32, tag=f"rhs3{b}")
                    for pss, gm_ap in ((0, gm_sb), (1, gm2_sb)):
                        nc.vector.tensor_tensor(
                            out=rhs3[:, 32 * pss : 32 * pss + 16],
                            in0=nv64[:].to_broadcast([S, NTILES]),
                            in1=gm_ap[:],
                            op=Alu.mult,
                        )
                        nc.vector.tensor_tensor(
                            out=rhs3[:, 32 * pss + 16 : 32 * pss + 32],
                            in0=start[:].to_broadcast([S, NTILES]),
                            in1=gm_ap[:],
                            op=Alu.mult,
                        )
                    bmat_ps = ppool.tile([128, 4 * NTILES], FP32, tag=f"bmat{b}")
                    nc.tensor.matmul(
                        out=bmat_ps[:, 0:32], lhsT=sel_sb[:], rhs=rhs3[:, 0:32],
                        start=True, stop=True,
                    )
                    nc.tensor.matmul(
                        out=bmat_ps[:, 32:64], lhsT=sel2_sb[:], rhs=rhs3[:, 32:64],
                        start=True, stop=True,
                    )
                    for pss in (0, 1):
                        nvB = bmat_ps[:, 32 * pss : 32 * pss + 16]
                        stB = bmat_ps[:, 32 * pss + 16 : 32 * pss + 32]
                        jt = spool.tile([128, NTILES], FP32, tag=f"jt{b}{pss}")
                        nc.vector.tensor_tensor(
                            out=jt[:], in0=tp_sb[:], in1=stB, op=Alu.subtract
                        )
                        vpos = spool.tile([128, NTILES], FP32, tag=f"vp{b}{pss}")
                        nc.vector.tensor_scalar(
                            out=vpos[:], in0=jt[:], scalar1=0.0, scalar2=None,
                            op0=Alu.is_ge,
                        )
                        v = spool.tile([128, NTILES], FP32, tag=f"v{b}{pss}")
                        nc.vector.tensor_tensor(
                            out=v[:], in0=jt[:], in1=nvB, op=Alu.is_lt
                        )
                        nc.vector.tensor_tensor(
                            out=v[:], in0=v[:], in1=vpos[:], op=Alu.mult
                        )
                        # outrow = srow1 - 64*pass + j (batch-row-local)
                        outr = spool.tile([128, NTILES], FP32, tag=f"or{b}{pss}")
                        nc.vector.tensor_tensor(
                            out=outr[:], in0=jt[:], in1=sr_sb[:], op=Alu.add
                        )
                        shift = float(-64 * pss)
                        if shift != 0.0:
                            nc.vector.tensor_scalar(
                                out=outr[:], in0=outr[:], scalar1=shift,
                                scalar2=None, op0=Alu.add,
                            )
                        # invalid slots -> 99999 (> bounds_check, skipped)
                        nc.vector.tensor_scalar(
                            out=outr[:], in0=outr[:], scalar1=-99999.0,
                            scalar2=None, op0=Alu.add,
                        )
                        nc.vector.tensor_tensor(
                            out=outr[:], in0=outr[:], in1=v[:], op=Alu.mult
                        )
                        nc.vector.tensor_scalar(
                            out=outr[:], in0=outr[:], scalar1=99999.0,
                            scalar2=None, op0=Alu.add,
                        )
                        sidx = spool.tile([128, NTILES], I32, tag=f"sidx{b}{pss}")
                        nc.vector.tensor_copy(out=sidx[:], in_=outr[:])
                        if variant == "scatterdbg":
                            nc.sync.dma_start(
                                out=dbg[
                                    :,
                                    (2 * b + pss) * NTILES : (2 * b + pss + 1) * NTILES,
                                ],
                                in_=sidx[:],
                            )
                        nc.gpsimd.indirect_dma_start(
                            out=outs[b],
                            out_offset=IndirectOffsetOnAxis(ap=sidx[:], axis=0),
                            in_=xbig[:, b * NTILES * E : (b + 1) * NTILES * E],
                            in_offset=None,
                            bounds_check=T - 1,
                            oob_is_err=False,
                        )
                    continue

                # ---- phase 4: broadcast start/nvalid to the 128 partitions of
                # each output group via one matmul against selmat ----
                # rhs [S, 2*NGROUPS]: cols [0, NG) = nv * gmask, cols [NG, 2NG) = start * gmask
                rhs2 = spool.tile([S, 2 * NGROUPS], FP32, tag=f"rhs2{b}")
                nc.vector.tensor_tensor(
                    out=rhs2[:, :NGROUPS],
                    in0=nv[:].to_broadcast([S, NGROUPS]),
                    in1=gm_sb[:],
                    op=Alu.mult,
                )
                nc.vector.tensor_tensor(
                    out=rhs2[:, NGROUPS:],
                    in0=start[:].to_broadcast([S, NGROUPS]),
                    in1=gm_sb[:],
                    op=Alu.mult,
                )
                bmat_ps = ppool.tile([128, 2 * NGROUPS], FP32, tag=f"bmat{b}")
                nc.tensor.matmul(
                    out=bmat_ps[:], lhsT=sel_sb[:], rhs=rhs2[:], start=True, stop=True
                )

                # mask[p, g] = (p % 64) < nv_broadcast
                mask = spool.tile([128, NGROUPS], FP32, tag=f"mask{b}")
                nc.vector.tensor_tensor(
                    out=mask[:],
                    in0=jp_sb[:].to_broadcast([128, NGROUPS]),
                    in1=bmat_ps[:, :NGROUPS],
                    op=Alu.is_lt,
                )
                # gidx[p, g] = b*T + start_broadcast + (p % 64), as int32
                gidx_f = spool.tile([128, NGROUPS], FP32, tag=f"gidxf{b}")
                nc.vector.tensor_tensor(
                    out=gidx_f[:],
                    in0=jp_sb[:].to_broadcast([128, NGROUPS]),
                    in1=bmat_ps[:, NGROUPS:],
                    op=Alu.add,
                )
                if b > 0:
                    nc.vector.tensor_scalar(
                        out=gidx_f[:], in0=gidx_f[:], scalar1=float(b * T),
                        scalar2=None, op0=Alu.add,
                    )
                gidx = spool.tile([128, NGROUPS], I32, tag=f"gidx{b}")
                nc.vector.tensor_copy(out=gidx[:], in_=gidx_f[:])
                if upto <= 4:
                    continue

                # ---- phase 5: gather + mask + store, 128 rows per group ----
                for g in range(NGROUPS):
                    gt = gpool.tile([128, E], FP32, tag="gt")
                    if variant == "nogather":
                        nc.sync.dma_start(
                            out=gt[:],
                            in_=x[b * T + g * 128 : b * T + (g + 1) * 128, :],
                        )
                    else:
                        nc.gpsimd.indirect_dma_start(
                            out=gt[:],
                            out_offset=None,
                            in_=x,
                            in_offset=IndirectOffsetOnAxis(
                                ap=gidx[:, g : g + 1], axis=0
                            ),
                        )
                    if variant != "nomask":
                        nc.vector.tensor_scalar(
                            out=gt[:], in0=gt[:], scalar1=mask[:, g : g + 1],
                            scalar2=None, op0=Alu.mult,
                        )
                    nc.sync.dma_start(
                        out=out[b * T + g * 128 : b * T + (g + 1) * 128, :], in_=gt[:]
                    )

    nc.compile()
    return nc


DEFAULT_VARIANT = "full"

_NC = {}


def _get_nc(variant: str = DEFAULT_VARIANT):
    if variant not in _NC:
        _NC[variant] = build_nc(debug=False, variant=variant)
    return _NC[variant]


def out_names(variant: str = DEFAULT_VARIANT):
    if variant.startswith("scat"):
        return [f"out{b}" for b in range(B_LOC)]
    return ["out"]


def assemble_out(results, variant: str = DEFAULT_VARIANT) -> np.ndarray:
    names = out_names(variant)
    # scat2 outputs carry TRASH spare rows beyond T; drop them.
    parts = [np.concatenate([r[n][:T] for n in names], axis=0) for r in results]
    return np.stack(parts).reshape(B, S, L, E)


def make_in_maps(inputs: np.ndarray, W: np.ndarray, b: np.ndarray):
    (
        selmat, gmask, jpat, basev, invt, kmat, tpat, srow1, sel2, gmask2,
        tpat16, srow16, trash16, id32,
    ) = _host_constants()
    W = np.ascontiguousarray(W, dtype=np.float32)
    b_s = np.ascontiguousarray(b[:S].reshape(S, 1), dtype=np.float32)
    b_e = np.ascontiguousarray(b[S:].reshape(S, 1), dtype=np.float32)
    in_maps = []
    for k in range(NCORES):
        xk = np.ascontiguousarray(
            inputs[k * B_LOC : (k + 1) * B_LOC].reshape(ROWS, E), dtype=np.float32
        )
        in_maps.append(
            {
                "x": xk,
                "w": W,
                "b_s": b_s,
                "b_e": b_e,
                "selmat": selmat,
                "gmask": gmask,
                "jpat": jpat,
                "basev": basev,
                "invt": invt,
                "kmat": kmat,
                "tpat": tpat,
                "srow1": srow1,
                "sel2": sel2,
                "gmask2": gmask2,
                "tpat16": tpat16,
                "srow16": srow16,
                "trash16": trash16,
                "id32": id32,
            }
        )
    return in_maps


def kernel(inputs: np.ndarray, W: np.ndarray, b: np.ndarray) -> np.ndarray:
    nc = _get_nc()
    in_maps = make_in_maps(np.asarray(inputs), np.asarray(W), np.asarray(b))
    res = bass_utils.run_bass_kernel_spmd(nc, in_maps, core_ids=list(range(NCORES)))
    return assemble_out(res.results)
